# revision 10
# baseline (speedup 1.0000x reference)
"""Causal self-attention (muP) for Trainium2, 8 NeuronCores, v3 (97.6us).

v2 baseline (107.1us) + scheduling round:
 - startup: tile-major x packing + fc-major wq/wk packing so the critical
   DMA chain to the first exp is [wq_fc0, wk_fc0, x8_0(a,b), cpk]; vproj(0)
   and qkproj(0,fc1) demoted to pipeline fillers.
 - S matmuls emitted under tc.high_priority so the exp stream's PSUM supply
   preempts filler work on PE.
 - tail: PV LAG tapers to 1 over the last head; the last head's normalize +
   row-parallel proj half runs per-qb as soon as that qb's column window has
   its final PV write (PSUM sub-range deps), with evacuations on the
   then-idle Act engine.

Everything else (dtype/engine plan, mask-bias matmuls, pair-batched exp,
ones-column Z trick) is unchanged from v2; see that docstring.
"""

import os
import sys

for _p in ("/opt/trn_rl_repo",):
    if _p not in sys.path:
        sys.path.insert(0, _p)

import numpy as np
import ml_dtypes

import concourse.bass as bass  # noqa: F401
import concourse.mybir as mybir
import concourse.tile as tile
from concourse import bacc
from concourse.bass_utils import run_bass_kernel_spmd
from concourse.tile import ScopedClock

# ---- problem constants (hardcoded per contract) ----
B, T, C = 2, 2048, 1024
NH, DH = 16, 64
N_CORES = 8
GROUPS = 4                 # head groups (tensor parallel)
NH_LOC = NH // GROUPS      # 4 heads per core
F = NH_LOC * DH            # 256 per-core qkv features
P = 128
CC = C // P                # 8 contraction chunks over C
TQ = 512                   # query tile width
NJ = T // TQ               # 4 query tiles
QB = TQ // P               # 4 query blocks per tile
NTC = T // P               # 16 key blocks
FC = F // P                # 2 feature chunks (head pairs)
f32 = mybir.dt.float32
bf16 = mybir.dt.bfloat16
e4 = mybir.dt.float8e4
e5 = mybir.dt.float8e5
EXP = mybir.ActivationFunctionType.Exp
DR = mybir.MatmulPerfMode.DoubleRow
WSCALE = 8.0               # host prescale on wq/wk before fp8 cast
SSCALE = 1.0 / (WSCALE * WSCALE * DH)   # exp scale: S_psum = 64*q.k
MASK_NEG = -57344.0        # e5m2 max; x64 via idT5 -> exp underflows to 0

# Quartic softmax-exp offload: for fully-below-diagonal S pairs, exp(x) is
# replaced (up to the softmax-invariant 1/24 scale) by the exact real
# factorization 24*exp4(x) = (x^2+ax+b)(x^2+cx+d) evaluated on DVE (2 bf16
# stt passes off a single PSUM read) + Pool (ts2/ts/tt), freeing ~1us of Act
# time per offloaded slot. Constants are pre-scaled for X = 4096*x.
PSC = 1.0 / SSCALE         # 4096: psum units per logit
P_A1 = PSC * 0.5411115379
P_A2 = PSC * 3.4588884621
P_B = PSC * PSC * 6.3471027552
P_D = PSC * PSC * 3.7812527898
P_S = 1.0 / (24.0 * PSC ** 4)
POLY = set()


def _install_drain_patch():
    """This walrus build rejects >2 sem waits on a single instruction; the
    Tile tail drain accumulates one wait per live proc. Split them into
    single-wait SP nops ahead of the drain."""
    if getattr(tile.TileContext, "_drain_patch_installed", False):
        return

    def _patched(self, tick_clock, wait_clock):
        nc = self.nc
        probe = nc.sync.nop(nofuse=True)
        wait_clock.add_sem_waits(
            probe.ins, ScopedClock({None: tick_clock.global_clock})
        )
        si = probe.ins.sync_info
        waits = list(si.on_wait) if si is not None and si.on_wait else []
        if len(waits) > 1:
            probe.ins.sync_info.on_wait = [waits[0]]
            for w in waits[1:]:
                n2 = nc.sync.nop(nofuse=True)
                n2.ins.sync_info = mybir.SyncInfo(on_wait=[w], on_update=[])
        nc.sync.drain()
        nc.all_engine_barrier()
        assert self.sems is not None
        popped = nc._tile_sem_poison_stack.pop()
        assert popped is self._sem_poison
        nc.clear_and_free_semaphores(list(self.sems.allocated().values()))
        nc.all_engine_barrier()

    tile.TileContext._drain_and_barrier = _patched
    tile.TileContext._drain_patch_installed = True


def build_module():
    _install_drain_patch()
    nc = bacc.Bacc("TRN2", target_bir_lowering=False, debug=False)
    xt16 = nc.dram_tensor("xt16", [P, CC * T], bf16, kind="ExternalInput").ap()
    xt8 = nc.dram_tensor("xt8", [P, CC * T], e4, kind="ExternalInput").ap()
    wqk8 = nc.dram_tensor("wqk8", [P, 2 * CC * F], e4, kind="ExternalInput").ap()
    wv16 = nc.dram_tensor("wv16", [P, CC * F], bf16, kind="ExternalInput").ap()
    wp16 = nc.dram_tensor("wp16", [P, FC * C], bf16, kind="ExternalInput").ap()
    cpk = nc.dram_tensor("cpk", [P, 1792], mybir.dt.uint8, kind="ExternalInput").ap()
    out = nc.dram_tensor("out", [T, C], bf16, kind="ExternalOutput").ap()
    out2 = nc.dram_tensor("out2", [TQ, C], bf16, kind="ExternalOutput").ap()
    out3 = nc.dram_tensor("out3", [TQ, C], bf16, kind="ExternalOutput").ap()
    out4 = nc.dram_tensor("out4", [TQ, C], bf16, kind="ExternalOutput").ap()

    with tile.TileContext(nc) as tc:
        _body(tc, xt16, xt8, wqk8, wv16, wp16, cpk, out, out2, out3, out4)
    nc.compile()
    return nc


def _body(tc, xt16, xt8, wqk8, wv16, wp16, cpk, out, out2, out3, out4):
    from contextlib import ExitStack, nullcontext

    nc = tc.nc
    with ExitStack() as ctx:
        const = ctx.enter_context(tc.tile_pool(name="const", bufs=1))
        wpool = ctx.enter_context(tc.tile_pool(name="wpool", bufs=1))
        qkv = ctx.enter_context(tc.tile_pool(name="qkv", bufs=1))
        xtp = ctx.enter_context(tc.tile_pool(name="xtp", bufs=3))
        sexp = ctx.enter_context(tc.tile_pool(name="sexp", bufs=12))
        ppool = ctx.enter_context(tc.tile_pool(name="ppool", bufs=2))
        ypool = ctx.enter_context(tc.tile_pool(name="ypool", bufs=2))
        small = ctx.enter_context(tc.tile_pool(name="small", bufs=8))
        outp = ctx.enter_context(tc.tile_pool(name="outp", bufs=4))
        ps_s1 = ctx.enter_context(tc.tile_pool(name="ps_s1", bufs=2, space="PSUM"))
        ps_sp = ctx.enter_context(tc.tile_pool(name="ps_sp", bufs=2, space="PSUM"))
        ps_y = ctx.enter_context(tc.tile_pool(name="ps_y", bufs=2, space="PSUM"))

        # tile-major x: [p, j, cc, tq]; merged q/k weights, g = 2*fc + which:
        # [p, g, cc, k] so the fc0 pair is a single small startup DMA
        xr16 = xt16.rearrange("p (j cc t) -> p j cc t", j=NJ, cc=CC)
        xr8 = xt8.rearrange("p (j cc t) -> p j cc t", j=NJ, cc=CC)
        wqkr = wqk8.rearrange("p (g cc k) -> p g cc k", g=4, cc=CC)

        # ---- persistent q^T/k^T (fp8, zero second k-tile slot) and V ----
        qT8 = [qkv.tile([P, 2, T], e4, name=f"qT8_{fc}") for fc in range(FC)]
        kT8 = [qkv.tile([P, 2, T], e4, name=f"kT8_{fc}") for fc in range(FC)]

        def zero_slot1(j):
            # S(j) reads slot 1 of kT8 cols [j*TQ,(j+1)*TQ) (lhsT) and of
            # qT8's j window (rhs); zero them one tile ahead on Pool (SBUF
            # memsets are legal there, and it keeps DVE free for evacuations)
            for fc in range(FC):
                nc.gpsimd.memset(kT8[fc][:, 1, j * TQ:(j + 1) * TQ], 0.0)
                nc.gpsimd.memset(qT8[fc][:, 1, j * TQ:(j + 1) * TQ], 0.0)

        zero_slot1(0)
        Vp = qkv.tile([P, NTC, NH_LOC * (DH + 1)], bf16)
        for h in range(NH_LOC):
            nc.gpsimd.memset(Vp[:, :, h * (DH + 1) + DH], 1.0)

        # ---- DMA order = need order: the first exp needs the fc0 q/k
        # weights, x8 tile 0 and the mask consts; everything else trails in
        # first-use order (wv+x16_0 for vproj(0), x8_1/x16_1 for tile-1
        # fillers, wp for the first proj).
        wqkT = wpool.tile([P, 4, CC, P], e4)
        wvT = wpool.tile([P, CC, F], bf16)
        wpT = wpool.tile([P, FC, C], bf16)
        cpkt = const.tile([P, 1792], mybir.dt.uint8)
        x8_0 = xtp.tile([P, CC, TQ], e4, tag="x8", name="x8_0")
        nc.sync.dma_start(out=wqkT[:, 0:2], in_=wqkr[:, 0:2])
        nc.sync.dma_start(out=x8_0, in_=xr8[:, 0])
        nc.sync.dma_start(out=cpkt, in_=cpk)
        nc.sync.dma_start(out=wqkT[:, 2:4], in_=wqkr[:, 2:4])
        x16_0 = xtp.tile([P, CC, TQ], bf16, tag="x16", name="x16_0")
        nc.sync.dma_start(out=wvT, in_=wv16.rearrange("p (cc f) -> p cc f", cc=CC))
        nc.sync.dma_start(out=x16_0, in_=xr16[:, 0])
        x8_1 = xtp.tile([P, CC, TQ], e4, tag="x8", name="x8_1")
        nc.sync.dma_start(out=x8_1, in_=xr8[:, 1])
        x16_1 = xtp.tile([P, CC, TQ], bf16, tag="x16", name="x16_1")
        nc.sync.dma_start(out=x16_1, in_=xr16[:, 1])
        nc.sync.dma_start(out=wpT, in_=wp16.rearrange("p (fc c) -> p fc c", fc=FC))
        # head-3 rows of the fb1 proj weights at base partition 0, so the
        # tail's 64-row-contraction matmuls line up with the y^T evacuation
        wph3 = wpool.tile([DH, C], bf16)
        nc.sync.dma_start(
            out=wph3,
            in_=wp16.rearrange("p (fc c) -> p fc c", fc=FC)[64:128, 1, :],
        )
        wph1 = wpool.tile([DH, C], bf16)
        nc.sync.dma_start(
            out=wph1,
            in_=wp16.rearrange("p (fc c) -> p fc c", fc=FC)[64:128, 0, :],
        )

        # packed consts: bitcast read-only views
        ubT5 = cpkt[:, 0:256].bitcast(e5).rearrange("p (s q) -> p s q", s=2)
        idT5 = cpkt[:, 256:512].bitcast(e5).rearrange("p (s q) -> p s q", s=2)
        ubB5 = cpkt[:, 512:768].bitcast(e5).rearrange("p (s q) -> p s q", s=2)
        idB5 = cpkt[:, 768:1280].bitcast(e5).rearrange(
            "p (s q) -> p s q", s=2
        )
        identb = cpkt[:, 1280:1536].bitcast(bf16)

        xs = {0: (x8_0, x16_0), 1: (x8_1, x16_1)}

        def load_x(j):
            x8 = xtp.tile([P, CC, TQ], e4, tag="x8", name=f"x8_{j}")
            nc.sync.dma_start(out=x8, in_=xr8[:, j])
            x16 = xtp.tile([P, CC, TQ], bf16, tag="x16", name=f"x16_{j}")
            nc.sync.dma_start(out=x16, in_=xr16[:, j])
            xs[j] = (x8, x16)

        def qkproj_group(j, fc, which):
            # one [128 feats, 512] DoubleRow group + fp8 evacuation
            dst = qT8 if which == 0 else kT8
            x8 = xs[j][0]
            # tile-1's projections sit on the exp-stream critical path
            # (x8_1 lands mid-tile-0, behind vproj(0) in the PE queue)
            pq = ps_s1.tile([P, 512], f32, tag="s1", name=f"pq_{j}_{fc}_{which}")
            with tc.high_priority(5 * 10**5) if j == 1 else nullcontext():
                for m in range(CC // 2):
                    nc.tensor.matmul(
                        pq,
                        lhsT=wqkT[:, 2 * fc + which, 2 * m:2 * m + 2, :],
                        rhs=x8[:, 2 * m:2 * m + 2, :],
                        start=(m == 0),
                        stop=(m == CC // 2 - 1),
                        perf_mode=DR,
                    )
                nc.vector.tensor_copy(
                    dst[fc][:, 0, j * TQ:(j + 1) * TQ], pq
                )

        def vproj_group(j, r):
            x16 = xs[j][1]
            pv = ps_s1.tile([P, 512], f32, tag="s1", name=f"pv_{j}_{r}")
            for cc in range(CC):
                nc.tensor.matmul(
                    pv[:, 0:F],
                    lhsT=x16[:, cc, r * P:(r + 1) * P],
                    rhs=wvT[:, cc, :],
                    start=(cc == 0),
                    stop=(cc == CC - 1),
                )
            nc.vector.tensor_copy(
                Vp[:, 4 * j + r].rearrange("p (h c) -> p h c", c=DH + 1)[
                    :, :, 0:DH
                ],
                pv[:, 0:F],
            )

        def transpose_group(j, ysb_t, yts):
            # 8 PE transposes of [128,128] bf16 blocks; all 8 fit in one
            # PSUM bank via the bf16 bitcast view, then strided evacuations
            pt = ps_s1.tile([P, 512], f32, tag="s1", name=f"pt_{j}")
            ptv = pt.bitcast(bf16)
            for k in range(8):
                qb, fb = k // FC, k % FC
                nc.tensor.matmul(
                    ptv[:, k * P:(k + 1) * P],
                    lhsT=ysb_t[:, qb, fb * P:(fb + 1) * P],
                    rhs=identb,
                    is_transpose=True,
                    start=(k == 0),
                    stop=True,
                    skip_group_check=True,
                )
            ptr = ptv.rearrange("p (qb fb q) -> p qb fb q", fb=FC, q=P)
            for fb in range(FC):
                nc.vector.tensor_copy(
                    yts[fb].rearrange("p (qb q) -> p qb q", q=P),
                    ptr[:, :, fb, :],
                )

        def proj_group(j, yts, qb, chunked_dma=False):
            # out[qb] = y[qb] @ wproj.T (row-parallel partial), bf16 wire
            ob = outp.tile([P, C], bf16, tag="ob", name=f"ob_{j}_{qb}")
            rows = slice(j * TQ + qb * P, j * TQ + (qb + 1) * P)
            for n in range(2):
                po = ps_s1.tile([P, 512], f32, tag="s1", name=f"po_{j}_{qb}_{n}")
                for fb in range(FC):
                    nc.tensor.matmul(
                        po,
                        lhsT=yts[fb][:, qb * P:(qb + 1) * P],
                        rhs=wpT[:, fb, n * 512:(n + 1) * 512],
                        start=(fb == 0),
                        stop=(fb == FC - 1),
                    )
                nc.vector.tensor_copy(ob[:, n * 512:(n + 1) * 512], po)
                nc.sync.dma_start(
                    out=out[rows, n * 512:(n + 1) * 512],
                    in_=ob[:, n * 512:(n + 1) * 512],
                )

        def emit_proj_tail_half0():
            # fb=0 half of the last tile's projection (heads 0/1), emitted
            # mid-tile so it hides under heads 2/3's exp stream. bf16 partial
            # to out2; the host sums the two row-parallel halves.
            for qb in range(QB):
                ob = outp.tile([P, C], bf16, tag="ob", name=f"obt_0_{qb}")
                for n in range(2):
                    po = ps_s1.tile(
                        [P, 512], f32, tag="s1", name=f"pot_0_{qb}_{n}"
                    )
                    nc.tensor.matmul(
                        po,
                        lhsT=yts_last[0][:, qb * P:(qb + 1) * P],
                        rhs=wpT[:, 0, n * 512:(n + 1) * 512],
                        start=True,
                        stop=True,
                    )
                    nc.vector.tensor_copy(ob[:, n * 512:(n + 1) * 512], po)
                    nc.sync.dma_start(
                        out=out2[
                            qb * P:(qb + 1) * P,
                            n * 512:(n + 1) * 512,
                        ],
                        in_=ob[:, n * 512:(n + 1) * 512],
                    )

        def emit_tail_h2():
            # head 2's half of the fb1 projection (64-row contraction), run
            # mid-tile right after norm(3,2); partial goes to `out` rows and
            # the host adds the head-3 partial from out3 on top
            for qb in range(QB):
                w = slice(qb * P, (qb + 1) * P)
                ob = outp.tile([P, C], bf16, tag="ob", name=f"obh2_{qb}")
                rows = slice(
                    (NJ - 1) * TQ + qb * P, (NJ - 1) * TQ + (qb + 1) * P
                )
                for n in range(2):
                    po = ps_s1.tile(
                        [P, 512], f32, tag="s1", name=f"poth2_{qb}_{n}"
                    )
                    nc.tensor.matmul(
                        po,
                        lhsT=yts_last[1][0:64, w],
                        rhs=wpT[0:64, 1, n * 512:(n + 1) * 512],
                        start=True,
                        stop=True,
                    )
                    # n=1 evacuations ride the Act queue: they land after the
                    # last exps drain, freeing DVE for the h3 tail chain
                    if n == 0:
                        nc.vector.tensor_copy(ob[:, n * 512:(n + 1) * 512], po)
                    else:
                        nc.scalar.copy(ob[:, n * 512:(n + 1) * 512], po)
                nc.sync.dma_start(out=out[rows, :], in_=ob)

        tail_state = {}

        def emit_tail_h3(half):
            # head 3 tail without a normalize chain: transpose the PSUM Z
            # row into per-partition scalars (Act copy -> tiny PE transposes
            # -> reciprocal), project the UNnormalized y^T and fold 1/Z into
            # the per-partition-scaled evacuations. Split in column halves:
            # qb 0/1's windows get their final PV write one slot earlier.
            pyf = py_tiles[(NJ - 1, NH_LOC - 1)].rearrange("p a b -> p (a b)")
            cols = slice(half * 256, half * 256 + 256)
            if half == 0:
                tail_state["zrow"] = small.tile(
                    [1, TQ], bf16, tag="zrow", name="zrow"
                )
                tail_state["yub"] = ypool.tile(
                    [DH, TQ], bf16, tag="yub", name="yub"
                )
                zq = ps_sp.tile([P, 2, TQ], f32, tag="sp", name="zq")
                tail_state["zqv"] = zq.rearrange(
                    "p a b -> p (a b)"
                ).bitcast(bf16).rearrange("p (c k) -> p c k", k=2)
                tail_state["rc"] = small.tile(
                    [P, QB], f32, tag="rct", name="rct"
                )
            zrow, yub = tail_state["zrow"], tail_state["yub"]
            zqv, rc = tail_state["zqv"], tail_state["rc"]
            nc.scalar.copy(zrow[:, cols], pyf[DH:DH + 1, cols])
            nc.vector.tensor_copy(yub[:, cols], pyf[0:DH, cols])
            for qb in (2 * half, 2 * half + 1):
                # even bf16 column slots keep the PSUM writes 4-byte aligned
                nc.tensor.matmul(
                    zqv[:, qb, 0:1],
                    lhsT=zrow[:, qb * P:(qb + 1) * P],
                    rhs=identb[0:1, 0:1],
                    is_transpose=True,
                    start=(qb == 2 * half),
                    stop=True,
                    skip_group_check=True,
                )
            nc.vector.reciprocal(
                rc[:, 2 * half:2 * half + 2], zqv[:, 2 * half:2 * half + 2, 0]
            )
            for qb in (2 * half, 2 * half + 1):
                w = slice(qb * P, (qb + 1) * P)
                ob = outp.tile([P, C], bf16, tag="ob", name=f"obt1_{qb}")
                rows = slice(qb * P, (qb + 1) * P)
                for n in range(2):
                    # alternate the drain PSUM between the s1 ring and the
                    # now-free S-pair banks so four pots pipeline in flight
                    if n == 0:
                        po = ps_s1.tile(
                            [P, 512], f32, tag="s1", name=f"pot1_{qb}_{n}"
                        )
                    else:
                        po = ps_sp.tile(
                            [P, 2, TQ], f32, tag="sp", name=f"pot1_{qb}_{n}"
                        ).rearrange("p a b -> p (a b)")[:, 0:512]
                    nc.tensor.matmul(
                        po,
                        lhsT=yub[:, w],
                        rhs=wph3[:, n * 512:(n + 1) * 512],
                        start=True,
                        stop=True,
                    )
                    if n == 0:
                        nc.vector.tensor_scalar_mul(
                            ob[:, n * 512:(n + 1) * 512], po, rc[:, qb:qb + 1]
                        )
                    else:
                        nc.scalar.activation(
                            ob[:, n * 512:(n + 1) * 512], po,
                            mybir.ActivationFunctionType.Copy,
                            scale=rc[:, qb:qb + 1],
                        )
                nc.sync.dma_start(out=out3[rows, :], in_=ob)

        tail1_state = {}

        def emit_tail_h1(half):
            # final head (order [2,3,0,1]): h1's fb0 half, unnormalized y^T
            # projected against base-0 wph1 with 1/Z folded into the
            # evacuations (DVE/Act split -- Act is idle post-stream)
            pyf = py_tiles[(NJ - 1, 1)].rearrange("p a b -> p (a b)")
            cols = slice(half * 256, half * 256 + 256)
            if half == 0:
                tail1_state["zrow"] = small.tile(
                    [1, TQ], bf16, tag="zrow", name="zrow1"
                )
                tail1_state["yub"] = ypool.tile(
                    [DH, TQ], bf16, tag="yub", name="yub1"
                )
                zq = ps_sp.tile([P, 2, TQ], f32, tag="sp", name="zq1")
                tail1_state["zqv"] = zq.rearrange(
                    "p a b -> p (a b)"
                ).bitcast(bf16).rearrange("p (c k) -> p c k", k=2)
                tail1_state["rc"] = small.tile(
                    [P, QB], f32, tag="rct", name="rct1"
                )
            zrow, yub = tail1_state["zrow"], tail1_state["yub"]
            zqv, rc = tail1_state["zqv"], tail1_state["rc"]
            nc.scalar.copy(zrow[:, cols], pyf[DH:DH + 1, cols])
            nc.vector.tensor_copy(yub[:, cols], pyf[0:DH, cols])
            for qb in (2 * half, 2 * half + 1):
                nc.tensor.matmul(
                    zqv[:, qb, 0:1],
                    lhsT=zrow[:, qb * P:(qb + 1) * P],
                    rhs=identb[0:1, 0:1],
                    is_transpose=True,
                    start=(qb == 2 * half),
                    stop=True,
                    skip_group_check=True,
                )
            nc.vector.reciprocal(
                rc[:, 2 * half:2 * half + 2], zqv[:, 2 * half:2 * half + 2, 0]
            )
            for qb in (2 * half, 2 * half + 1):
                w = slice(qb * P, (qb + 1) * P)
                ob = outp.tile([P, C], bf16, tag="ob", name=f"obt4_{qb}")
                rows = slice(qb * P, (qb + 1) * P)
                for n in range(2):
                    if n == 0:
                        po = ps_s1.tile(
                            [P, 512], f32, tag="s1", name=f"pot4_{qb}_{n}"
                        )
                    else:
                        po = ps_sp.tile(
                            [P, 2, TQ], f32, tag="sp", name=f"pot4_{qb}_{n}"
                        ).rearrange("p a b -> p (a b)")[:, 0:512]
                    nc.tensor.matmul(
                        po,
                        lhsT=yub[:, w],
                        rhs=wph1[:, n * 512:(n + 1) * 512],
                        start=True,
                        stop=True,
                    )
                    if n == 0:
                        nc.vector.tensor_scalar_mul(
                            ob[:, n * 512:(n + 1) * 512], po, rc[:, qb:qb + 1]
                        )
                    else:
                        nc.scalar.activation(
                            ob[:, n * 512:(n + 1) * 512], po,
                            mybir.ActivationFunctionType.Copy,
                            scale=rc[:, qb:qb + 1],
                        )
                nc.sync.dma_start(out=out4[rows, :], in_=ob)

        tail_state = {}

        def emit_tail_fb0():
            # tile-3 fb0: 4 PE transposes of the normalized ysb columns into
            # y^T form, one evacuation, then the fb0 proj half (hidden under
            # heads 2/3's exp stream)
            ysb3 = ysb_tiles[NJ - 1]
            pt = ps_s1.tile([P, 512], f32, tag="s1", name="ptf0")
            ptv = pt.bitcast(bf16)
            for qb in range(QB):
                nc.tensor.matmul(
                    ptv[:, qb * P:(qb + 1) * P],
                    lhsT=ysb3[:, qb, 0:P],
                    rhs=identb,
                    is_transpose=True,
                    start=(qb == 0),
                    stop=True,
                    skip_group_check=True,
                )
            nc.vector.tensor_copy(yts_last[0], ptv[:, 0:TQ])
            emit_proj_tail_half0()

        def emit_tail_fb1(half):
            # tile-3 fb1 tail in PV-slot halves: qb 0/1's denominators and y
            # columns take their final PSUM write one PV slot earlier (their
            # diagonal stops are in slot m=6), so their normalize + chain
            # starts before the last exp drains. Per qb: transpose -> evac ->
            # proj -> split evacuations (Act idle post-stream) -> DMA; the
            # second pot borrows the freed S-pair PSUM ring.
            py33 = py_tiles[(NJ - 1, NH_LOC - 1)]
            ysb3 = ysb_tiles[NJ - 1]
            rc33 = tail_state.setdefault(
                "rc33", small.tile([P, QB], f32, tag="rc", name="rc_3_3")
            )
            nc.vector.reciprocal(
                rc33[:, 2 * half:2 * half + 2],
                py33[:, 2 * half:2 * half + 2, DH:DH + 1],
            )
            for qb in (2 * half, 2 * half + 1):
                nc.vector.tensor_scalar_mul(
                    ysb3[:, qb, 3 * DH:4 * DH],
                    py33[:, qb, 0:DH],
                    rc33[:, qb:qb + 1],
                )
            # both qbs' transposes into ONE psum tile and a single strided
            # evacuation, so the proj matmuls pipeline behind one DVE pass
            pt = ps_s1.tile([P, 512], f32, tag="s1", name=f"ptf1_{half}")
            ptv = pt.bitcast(bf16)
            for qb in (2 * half, 2 * half + 1):
                nc.tensor.matmul(
                    ptv[:, (qb % 2) * P:(qb % 2) * P + P],
                    lhsT=ysb3[:, qb, P:2 * P],
                    rhs=identb,
                    is_transpose=True,
                    start=(qb == 2 * half),
                    stop=True,
                    skip_group_check=True,
                )
            nc.vector.tensor_copy(
                yts_last[1][:, half * 256:half * 256 + 256], ptv[:, 0:256]
            )
            for qb in (2 * half, 2 * half + 1):
                ob = outp.tile([P, C], bf16, tag="ob", name=f"obt1_{qb}")
                rows = slice(
                    (NJ - 1) * TQ + qb * P, (NJ - 1) * TQ + (qb + 1) * P
                )
                for n in range(2):
                    if n == 0:
                        po = ps_s1.tile(
                            [P, 512], f32, tag="s1", name=f"pot1_{qb}_{n}"
                        )
                    else:
                        po = ps_sp.tile(
                            [P, 2, TQ], f32, tag="sp", name=f"pot1_{qb}_{n}"
                        ).rearrange("p a b -> p (a b)")[:, 0:512]
                    nc.tensor.matmul(
                        po,
                        lhsT=yts_last[1][:, qb * P:(qb + 1) * P],
                        rhs=wpT[:, 1, n * 512:(n + 1) * 512],
                        start=True,
                        stop=True,
                    )
                    if n == 0:
                        nc.vector.tensor_copy(ob[:, 0:512], po)
                    else:
                        nc.scalar.copy(ob[:, 512:1024], po)
                nc.sync.dma_start(out=out[rows, :], in_=ob)

        # prologue: only the head-0 projections inline; everything else is
        # filler work so the exp stream starts as soon as the DMAs land.
        # The k projection runs in column halves: S(0,0,0) only needs key
        # blocks 0/1, so its half evacuates a full group earlier.
        qkproj_group(0, 0, 0)
        for half in range(2):
            cols = slice(half * 256, half * 256 + 256)
            pqh = ps_s1.tile([P, 256], f32, tag="s1", name=f"pqk0_{half}")
            for m in range(CC // 2):
                nc.tensor.matmul(
                    pqh,
                    lhsT=wqkT[:, 1, 2 * m:2 * m + 2, :],
                    rhs=x8_0[:, 2 * m:2 * m + 2, cols],
                    start=(m == 0),
                    stop=(m == CC // 2 - 1),
                    perf_mode=DR,
                )
            nc.vector.tensor_copy(kT8[0][:, 0, cols], pqh)

        # ---- one global cross-tile slot pipeline ----
        # Tiles 0-2 use block-major slots ("B", j, fc, i): one key block i
        # for BOTH heads of the fc pair. The two heads share the same causal
        # window, so every exp batch shrinks to the per-block minimum
        # (saves 256 cols per tile/fc vs pairing consecutive blocks of one
        # head) at the same instruction count. Tile 3 keeps the head-major
        # pair slots ("P", 3, h, m): its tail machinery needs heads to
        # finish sequentially.
        all_slots = []
        for j in range(NJ - 1):
            for fc in range(FC):
                for i in range(4 * j + 4):
                    all_slots.append(("B", j, fc, i))
        for h in range(NH_LOC):
            for m in range(2 * (NJ - 1) + 2):
                all_slots.append(("P", NJ - 1, h, m))
        NSL = len(all_slots)
        LAG = 5

        def lag_of(s):
            # taper to 1 over the last head so the tail drains immediately
            return min(LAG, max(1, NSL - 4 - s))

        ps_tiles = {}
        se_tiles = {}
        py_tiles = {}
        ysb_tiles = {}

        def emit_S(sl):
            if sl[0] == "B":
                _, j, fc, i = sl
                d = i - 4 * j
                st = max(0, 128 * d)
                ps = ps_sp.tile(
                    [P, 2, TQ], f32, tag="sp", name=f"sp_b{j}_{fc}_{i}"
                )
                ps_tiles[sl] = ps
                with tc.high_priority(10**6):
                    for s2 in range(2):
                        nc.tensor.matmul(
                            ps[:, s2, st:TQ],
                            lhsT=kT8[fc][
                                64 * s2:64 * (s2 + 1), :, i * P:(i + 1) * P
                            ],
                            rhs=qT8[fc][
                                64 * s2:64 * (s2 + 1), :,
                                j * TQ + st:(j + 1) * TQ
                            ],
                            start=True,
                            stop=(d < 0),
                            perf_mode=DR,
                        )
                        if d >= 0:
                            nc.tensor.matmul(
                                ps[:, s2, st:st + P],
                                lhsT=ubT5,
                                rhs=idT5,
                                start=False,
                                stop=True,
                                perf_mode=DR,
                            )
                return
            _, j, h, m = sl
            fc, h2 = h // 2, h % 2
            ps = ps_sp.tile(
                [P, 2, TQ], f32, tag="sp", name=f"sp_{j}_{h}_{m}"
            )
            ps_tiles[sl] = ps
            d0 = 2 * m - 4 * j
            st0 = max(0, 128 * d0)
            with tc.high_priority(10**6):
                for s2 in range(2):
                    i = 2 * m + s2
                    d = i - 4 * j
                    # both blocks of a pair share the window [st0, TQ) so the
                    # batched exp never reads unwritten psum. Diag blocks: mask
                    # bias goes FIRST (start=True zeroes the bank), S
                    # accumulates on top and closes the group; the second diag
                    # block's bias is the combined 256-wide [full | triangle].
                    nc.tensor.matmul(
                        ps[:, s2, st0:TQ],
                        lhsT=kT8[fc][
                            64 * h2:64 * (h2 + 1), :, i * P:(i + 1) * P
                        ],
                        rhs=qT8[fc][
                            64 * h2:64 * (h2 + 1), :, j * TQ + st0:(j + 1) * TQ
                        ],
                        start=True,
                        stop=(d < 0),
                        perf_mode=DR,
                    )
                    if d >= 0:
                        if s2 == 0:
                            nc.tensor.matmul(
                                ps[:, s2, st0:st0 + P],
                                lhsT=ubT5,
                                rhs=idT5,
                                start=False,
                                stop=True,
                                perf_mode=DR,
                            )
                        else:
                            nc.tensor.matmul(
                                ps[:, s2, st0:st0 + 2 * P],
                                lhsT=ubB5,
                                rhs=idB5,
                                start=False,
                                stop=True,
                                perf_mode=DR,
                            )

        def emit_exp(sl):
            if sl[0] == "B":
                _, j, fc, i = sl
                st = max(0, 128 * (i - 4 * j))
                se = sexp.tile(
                    [P, 2, TQ], bf16, tag="se", name=f"se_b{j}_{fc}_{i}"
                )
                se_tiles[sl] = se
                nc.scalar.activation(
                    se[:, :, st:TQ], ps_tiles.pop(sl)[:, :, st:TQ], EXP,
                    scale=SSCALE,
                )
                return
            _, j, h, m = sl
            se = sexp.tile(
                [P, 2, TQ], bf16, tag="se", name=f"se_{j}_{h}_{m}"
            )
            se_tiles[sl] = se
            st = max(0, 128 * (2 * m - 4 * j))
            ps = ps_tiles.pop(sl)
            if sl in POLY:
                # below-diagonal pair: quartic exp on DVE+Pool instead of Act
                add, mult = mybir.AluOpType.add, mybir.AluOpType.mult
                nm = f"{j}_{h}_{m}"
                xb = ppool.tile([P, 2, TQ], bf16, tag="xb", name=f"xb_{nm}")
                nc.vector.tensor_copy(xb, ps)  # single PSUM read, frees ring
                t1 = ppool.tile([P, 2, TQ], bf16, tag="t1", name=f"t1_{nm}")
                nc.vector.scalar_tensor_tensor(t1, xb, P_A1, xb, add, mult)
                t2 = ppool.tile([P, 2, TQ], bf16, tag="t2", name=f"t2_{nm}")
                nc.vector.scalar_tensor_tensor(t2, xb, P_A2, xb, add, mult)
                t3 = ppool.tile([P, 2, TQ], bf16, tag="t3", name=f"t3_{nm}")
                nc.gpsimd.tensor_scalar(t3, t2, P_D, P_S, add, mult)
                w1 = ppool.tile([P, 2, TQ], bf16, tag="w1", name=f"w1_{nm}")
                nc.gpsimd.tensor_scalar(w1, t1, P_B, None, add)
                nc.gpsimd.tensor_tensor(se, w1, t3, mult)
                return
            nc.scalar.activation(
                se[:, :, st:TQ], ps[:, :, st:TQ], EXP,
                scale=SSCALE,
            )

        def emit_PV(sl):
            if sl[0] == "B":
                _, j, fc, i = sl
                d = i - 4 * j
                if i == 0:
                    for s2 in range(2):
                        h = 2 * fc + s2
                        py_tiles[(j, h)] = ps_y.tile(
                            [P, QB, P], f32, tag="py", name=f"py_{j}_{h}"
                        )
                    if fc == 0:
                        # bf16: y rounds to bf16 at the transpose anyway, and
                        # a bf16 source halves the PE transpose cost
                        ysb_tiles[j] = ypool.tile(
                            [P, QB, F], bf16, tag="ysb", name=f"ysb_{j}"
                        )
                se = se_tiles.pop(sl)
                for s2 in range(2):
                    h = 2 * fc + s2
                    py = py_tiles[(j, h)]
                    for qb in range(QB):
                        if qb < d:
                            continue
                        nc.tensor.matmul(
                            py[:, qb, 0:DH + 1],
                            lhsT=se[:, s2, qb * P:(qb + 1) * P],
                            rhs=Vp[:, i, h * (DH + 1):(h + 1) * (DH + 1)],
                            start=(i == 0 and qb == 0),
                            stop=(i == 4 * j + qb),
                            skip_group_check=True,
                        )
                return
            _, j, h, m = sl
            if m == 0:
                py_tiles[(j, h)] = ps_y.tile(
                    [P, QB, P], f32, tag="py", name=f"py_{j}_{h}"
                )
                if h == 0:
                    ysb_tiles[j] = ypool.tile(
                        [P, QB, F], bf16, tag="ysb", name=f"ysb_{j}"
                    )
            py = py_tiles[(j, h)]
            se = se_tiles.pop(sl)
            for s2 in range(2):
                i = 2 * m + s2
                d = i - 4 * j
                for qb in range(QB):
                    if qb < d:
                        continue
                    nc.tensor.matmul(
                        py[:, qb, 0:DH + 1],
                        lhsT=se[:, s2, qb * P:(qb + 1) * P],
                        rhs=Vp[:, i, h * (DH + 1):(h + 1) * (DH + 1)],
                        start=(i == 0 and qb == 0),
                        stop=(i == 4 * j + qb),
                        skip_group_check=True,
                    )

        def emit_norm(j, h):
            py = py_tiles.pop((j, h))
            ysb = ysb_tiles[j]
            rc = small.tile([P, QB], f32, tag="rc", name=f"rc_{j}_{h}")
            nc.vector.reciprocal(rc, py[:, :, DH:DH + 1])
            for qb in range(QB):
                nc.vector.tensor_scalar_mul(
                    ysb[:, qb, h * DH:(h + 1) * DH],
                    py[:, qb, 0:DH],
                    rc[:, qb:qb + 1],
                )

        fillers = []          # drainable immediately
        fillers2 = []         # (ready_tile, fn, args): need norm(ready_tile, h3)
        fill_i = 0
        fill2_i = 0
        norm_done = set()
        yts_last = []

        # tile-0 extras that used to be prologue work
        fillers.append((qkproj_group, (0, 1, 0)))
        fillers.append((qkproj_group, (0, 1, 1)))
        for r in range(QB):
            fillers.append((vproj_group, (0, r)))

        def on_enter_tile(j):
            # fillers + next-but-one tile prep, queued as the exp pointer
            # crosses into tile j. transpose/proj of tile j-1 read ysb(j-1),
            # which is only complete once norm(j-1, h3) has been EMITTED --
            # gate them on that or the tile scheduler misses the dependency.
            # proj(j-1) is additionally deferred one more tile (min_tile) to
            # rebalance PE load toward the later, Act-heavier tiles.
            if j > 0:
                for r in range(QB):
                    fillers.append((vproj_group, (j, r)))
            if j + 1 < NJ:
                for fc in range(FC):
                    fillers.append((qkproj_group, (j + 1, fc, 0)))
                    fillers.append((qkproj_group, (j + 1, fc, 1)))
                if j >= 1:
                    load_x(j + 1)
                zero_slot1(j + 1)
            if j >= 1:
                yts_prev = [
                    ypool.tile(
                        [P, TQ], bf16, tag=f"yts{fb}", name=f"yts{fb}_{j-1}"
                    )
                    for fb in range(FC)
                ]
                fillers2.append(
                    (j - 1, j, transpose_group, (j - 1, ysb_tiles[j - 1], yts_prev))
                )
                for qb in range(QB):
                    fillers2.append(
                        (j - 1, min(j + 1, NJ - 1), proj_group,
                         (j - 1, yts_prev, qb))
                    )
            if j == NJ - 1:
                for fb in range(FC):
                    yts_last.append(
                        ypool.tile(
                            [P, TQ], bf16, tag=f"yts{fb}", name=f"yts{fb}_{j}"
                        )
                    )

        emit_S(all_slots[0])
        emit_S(all_slots[1])
        cur_tile = -1
        pv_i = 0
        for t in range(NSL + 1):
            if t < NSL:
                sl = all_slots[t]
                if sl[1] != cur_tile:
                    cur_tile = sl[1]
                    on_enter_tile(cur_tile)
                emit_exp(sl)
            while pv_i < NSL and pv_i + lag_of(pv_i) <= t:
                slp = all_slots[pv_i]
                emit_PV(slp)
                pv_i += 1
                if slp[0] == "B":
                    _, bj, bfc, bi = slp
                    if bi == 4 * bj + 3:
                        emit_norm(bj, 2 * bfc)
                        emit_norm(bj, 2 * bfc + 1)
                        if bfc == 1:
                            norm_done.add(bj)
                    continue
                last_head = slp[2] == NH_LOC - 1
                if last_head and slp[3] == 2 * slp[1]:
                    emit_tail_fb1(0)
                    # then emit the final PV so the second half follows at once
                    slp = all_slots[pv_i]
                    emit_PV(slp)
                    pv_i += 1
                if slp[3] == 2 * slp[1] + 1:
                    if last_head:
                        emit_tail_fb1(1)
                        py_tiles.pop((NJ - 1, NH_LOC - 1))
                        norm_done.add(slp[1])
                    else:
                        emit_norm(slp[1], slp[2])
                        if slp[2] == 1:
                            emit_tail_fb0()
            if t + 2 < NSL:
                emit_S(all_slots[t + 2])
            for _ in range(2):
                if fill_i < len(fillers):
                    fn, args = fillers[fill_i]
                    fn(*args)
                    fill_i += 1
                elif (
                    fill2_i < len(fillers2)
                    and fillers2[fill2_i][0] in norm_done
                    and cur_tile >= fillers2[fill2_i][1]
                ):
                    _, _, fn, args = fillers2[fill2_i]
                    fn(*args)
                    fill2_i += 1
        while fill_i < len(fillers):
            fn, args = fillers[fill_i]
            fn(*args)
            fill_i += 1
        while fill2_i < len(fillers2):
            _, _, fn, args = fillers2[fill2_i]
            fn(*args)
            fill2_i += 1


_CACHE = {}


def shard_inputs(x, wq, wk, wv, wproj):
    bf = ml_dtypes.bfloat16
    f8 = ml_dtypes.float8_e4m3
    f85 = ml_dtypes.float8_e5m2

    def arr_cc(a, dt):  # (C', F') -> [P, CC'*F'] with contraction = cc*P + p
        ccn = a.shape[0] // P
        return np.ascontiguousarray(
            a.reshape(ccn, P, a.shape[1]).transpose(1, 0, 2).reshape(P, -1)
        ).astype(dt)

    def arr_x(a, dt):  # (C, T) -> [P, NJ*CC*TQ] tile-major per partition
        return np.ascontiguousarray(
            a.reshape(CC, P, NJ, TQ).transpose(1, 2, 0, 3).reshape(P, -1)
        ).astype(dt)

    def arr_w(a, dt):  # (C, F) -> [P, FC*CC*128] fc-major per partition
        return np.ascontiguousarray(
            a.reshape(CC, P, FC, P).transpose(1, 2, 0, 3).reshape(P, -1)
        ).astype(dt)

    # causal bias consts: ub5[c, 0, k] = -57344 for k > c; id5 = 64*I slot 0
    ub = np.zeros((P, 2, P), np.float32)
    ub[:, 0, :] = np.triu(np.full((P, P), MASK_NEG, np.float32), 1)
    id_ = np.zeros((P, 2, P), np.float32)
    id_[:, 0, :] = np.eye(P, dtype=np.float32) * (WSCALE * WSCALE)
    ub5 = ub.reshape(P, 2 * P).astype(f85)
    id5 = id_.reshape(P, 2 * P).astype(f85)
    # second-of-diag-pair bias: out[k, n<128] = full, out[k, n>=128] = triangle
    ubb = np.zeros((P, 2, P), np.float32)
    ubb[:, 0, :] = MASK_NEG
    ubb[:, 1, :] = np.triu(np.full((P, P), MASK_NEG, np.float32), 1)
    idb2 = np.zeros((P, 2, 2 * P), np.float32)
    for i in range(2):
        for p in range(P):
            idb2[p, i, 128 * i + p] = WSCALE * WSCALE
    ubB = ubb.reshape(P, 2 * P).astype(f85)
    idB = idb2.reshape(P, 4 * P).astype(f85)
    idb = np.eye(P, dtype=np.float32).astype(bf)
    cpk = np.concatenate(
        [
            ub5.view(np.uint8),
            id5.view(np.uint8),
            ubB.view(np.uint8),
            idB.view(np.uint8),
            np.ascontiguousarray(idb).view(np.uint8),
            np.zeros((P, 256), np.uint8),
        ],
        axis=1,
    )

    in_maps = []
    for c in range(N_CORES):
        b, g = divmod(c, GROUPS)
        sl = slice(g * F, (g + 1) * F)
        xt = np.ascontiguousarray(x[b].T, dtype=np.float32)      # (C, T)
        wqh = arr_w(wq[sl, :].T * WSCALE, f8).reshape(P, FC, CC * P)
        wkh = arr_w(wk[sl, :].T * WSCALE, f8).reshape(P, FC, CC * P)
        # merged [p, g, cc*k] with g = (q-fc0, k-fc0, q-fc1, k-fc1)
        wqk = np.stack(
            [wqh[:, 0], wkh[:, 0], wqh[:, 1], wkh[:, 1]], axis=1
        ).reshape(P, -1)
        in_maps.append(
            {
                "xt16": arr_x(xt, bf),
                "xt8": arr_x(xt, f8),
                "wqk8": np.ascontiguousarray(wqk),
                "wv16": arr_cc(wv[sl, :].T, bf),
                "wp16": arr_cc(wproj[:, sl].T, bf),
                "cpk": cpk,
            }
        )
    return in_maps


def kernel(x, wq, wk, wv, wproj):
    x = np.asarray(x, dtype=np.float32)
    wq = np.asarray(wq, dtype=np.float32)
    wk = np.asarray(wk, dtype=np.float32)
    wv = np.asarray(wv, dtype=np.float32)
    wproj = np.asarray(wproj, dtype=np.float32)

    from concourse._compat import axon_active

    if axon_active():
        # the axon NTFF-profile hook isn't available in this environment;
        # a BASS_TRACE=1 run would crash importing it, so disable tracing
        os.environ.setdefault("BASS_NEVER_TRACE", "1")

    if "nc" not in _CACHE:
        _CACHE["nc"] = build_module()
    nc = _CACHE["nc"]

    in_maps = shard_inputs(x, wq, wk, wv, wproj)
    res = run_bass_kernel_spmd(nc, in_maps, core_ids=list(range(N_CORES)))
    outa = np.zeros((B, T, C), np.float32)
    for c in range(N_CORES):
        b = c // GROUPS
        outa[b] += res.results[c]["out"].astype(np.float32)
        outa[b, (NJ - 1) * TQ:] += res.results[c]["out2"].astype(np.float32)
        outa[b, (NJ - 1) * TQ:] += res.results[c]["out3"].astype(np.float32)
        outa[b, (NJ - 1) * TQ:] += res.results[c]["out4"].astype(np.float32)
    return outa
